# revision 16
# baseline (speedup 1.0000x reference)
"""Bahdanau-attention-with-coverage kernel for Trainium2 (Bass/Tile).

The reference module applies softmax over a size-1 axis, so every attention
weight is exactly 1.0. The whole network therefore collapses:
  context_vector     = values.sum(axis=1)               # [B, D]
  attention_weights  = ones([B, T, 1])
  coverage_vector    = arange(T) broadcast to [B, T, 1]
The kernel computes exactly that, data-parallel over batch across 8 cores.
Per core: sum-over-T of a [4, 1024, 1024] f32 slab -> DMA-bound (~16MB HBM
read at ~350 GB/s/core), with the fold work hidden under the DMA window:
  - 16x 1MB input DMAs, alternating the two HWDGE queues (SP / Activation)
  - per-chunk halves-fold (GpSimd for chunk0, DVE for chunks 1-3), then an
    arrival-ordered DVE tree; 128->1 partition fold on the PE into one PSUM
    tile (indicator-window weights route batch b to PSUM row b)
  - trivial outputs: memset(1.0) and gpsimd iota, via SWDGE
"""

import numpy as np

import concourse.bacc as bacc
import concourse.mybir as mybir
from concourse.tile import TileContext
from concourse.bass_utils import run_bass_kernel_spmd

B, T, D = 32, 1024, 1024
N_CORES = 8
BL = B // N_CORES  # batches per core
F32 = mybir.dt.float32
P = 128            # SBUF partitions
NCH = T // P       # 8 t-rows per partition

_nc = None


def _build():
    # TRN2 codegen allows only ONE sync-wait per instruction; the structure
    # below keeps every compute/DMA instruction at a single cross-engine
    # dependency (Bacc's event-semaphore pass legalizes the rest).
    nc = bacc.Bacc("TRN2", target_bir_lowering=False, num_swdge_queues=4)

    vals = nc.dram_tensor("values", [BL, T, D], F32, kind="ExternalInput")
    ctx_o = nc.dram_tensor("ctx", [BL, D], F32, kind="ExternalOutput")
    aw_o = nc.dram_tensor("aw", [BL, T, 1], F32, kind="ExternalOutput")
    cov_o = nc.dram_tensor("cov", [BL, T, 1], F32, kind="ExternalOutput")

    with TileContext(nc) as tc:
        with (
            tc.tile_pool(name="chunks", bufs=8) as chunks,
            tc.tile_pool(name="mid", bufs=2) as mid,
            tc.tile_pool(name="small", bufs=1) as small,
            tc.tile_pool(name="psum", bufs=1, space="PSUM") as psum,
        ):
            # attention_weights = 1.0 everywhere (emit first: runs during the
            # DMA-issue preamble on otherwise-idle engines)
            ones_t = small.tile([P, BL * T // P], F32)
            nc.vector.memset(ones_t[:], 1.0)
            nc.gpsimd.dma_start(
                out=aw_o.ap().flatten_outer_dims().rearrange("(p n) c -> p (n c)", p=P),
                in_=ones_t[:],
            )

            # coverage[b, t] = t;  layout t = NCH*p + n  ->  iota = NCH*p + n
            iota_t = small.tile([P, BL, NCH], mybir.dt.int32)
            nc.gpsimd.iota(
                iota_t[:], pattern=[[0, BL], [1, NCH]], base=0, channel_multiplier=NCH
            )
            cov_t = small.tile([P, BL, NCH], F32)
            nc.vector.tensor_copy(out=cov_t[:], in_=iota_t[:])
            nc.gpsimd.dma_start(
                out=cov_o.ap().rearrange("b (p n) c -> p b (n c)", p=P),
                in_=cov_t[:],
            )

            # Sliding-window indicator weights: wz[:, BL] == 1, else 0.  For
            # batch b, lhsT = wz[:, BL-b : 2*BL-b] is [128, BL] with column b
            # all ones -> the matmul routes batch b's partition-sum into PSUM
            # row b, accumulating all batches into ONE psum tile.
            wz = small.tile([P, 2 * BL], F32)
            nc.vector.memset(wz[:], 0.0)
            nc.vector.memset(wz[:, BL : BL + 1], 1.0)

            acc = psum.tile([BL, D], F32, tag="acc")

            # context[b] = sum_t values[b, t, :]
            dma_eng = [nc.sync, nc.scalar, nc.sync, nc.scalar]
            fold_eng = [nc.gpsimd, nc.vector, nc.vector, nc.vector]
            C = 2 * D  # input DMA chunk: [128, 2048] f32 = 1 MB
            for b in range(BL):
                src = vals.ap()[b].rearrange("(p n) d -> p (n d)", p=P)  # [128, 8192]

                # per-chunk halves-fold; each fold reads data from exactly ONE
                # dma_start (single sync-wait)
                f = []
                for i in range(4):
                    ck = chunks.tile([P, C], F32, tag="ck", name=f"ck{i}_{b}")
                    dma_eng[i].dma_start(out=ck[:], in_=src[:, i * C : (i + 1) * C])
                    fi = mid.tile(
                        [P, D], F32, tag=f"f{i}", name=f"f{i}_{b}",
                        bufs=4 if i == 0 else 2,
                    )
                    fold_eng[i].tensor_add(out=fi[:], in0=ck[:, :D], in1=ck[:, D:])
                    f.append(fi)

                # arrival-ordered tree on DVE: chunks 0/1 land first
                c01 = mid.tile([P, D], F32, tag="chain", bufs=6, name=f"c01_{b}")
                nc.vector.tensor_add(out=c01[:], in0=f[0][:], in1=f[1][:])
                c23 = mid.tile([P, D], F32, tag="chain", bufs=6, name=f"c23_{b}")
                nc.vector.tensor_add(out=c23[:], in0=f[2][:], in1=f[3][:])
                a1 = mid.tile([P, D], F32, tag="chain", bufs=6, name=f"a1_{b}")
                nc.vector.tensor_add(out=a1[:], in0=c01[:], in1=c23[:])

                # fold 128 partitions -> PSUM row b on the PE
                lhsT = wz[:, BL - b : 2 * BL - b]
                nc.tensor.matmul(
                    acc[:, 0:512], lhsT, a1[:, 0:512],
                    start=(b == 0), stop=(b == BL - 1),
                )
                nc.tensor.matmul(
                    acc[:, 512:1024], lhsT, a1[:, 512:1024],
                    start=(b == 0), stop=(b == BL - 1),
                )

            ctx_sb = small.tile([BL, D], F32)
            nc.vector.tensor_copy(out=ctx_sb[:], in_=acc[:])
            nc.gpsimd.dma_start(out=ctx_o.ap(), in_=ctx_sb[:])

    # legalizes multi-wait instructions (TRN2: max 1 sync-wait/instruction),
    # allocates registers, fuses nops
    nc.compile()
    return nc


def _get_nc():
    global _nc
    if _nc is None:
        _nc = _build()
    return _nc


def kernel(**inputs) -> tuple:
    values = np.ascontiguousarray(np.asarray(inputs["values"], dtype=np.float32))
    assert values.shape == (B, T, D), values.shape

    nc = _get_nc()
    in_maps = [
        {"values": np.ascontiguousarray(values[c * BL : (c + 1) * BL])}
        for c in range(N_CORES)
    ]
    res = run_bass_kernel_spmd(nc, in_maps, core_ids=list(range(N_CORES)))

    ctx = np.concatenate([r["ctx"] for r in res.results], axis=0)
    aw = np.concatenate([r["aw"] for r in res.results], axis=0)
    cov = np.concatenate([r["cov"] for r in res.results], axis=0)
    return ctx, aw, cov


# revision 17
# speedup vs baseline: 1.0593x; 1.0593x over previous
"""Bahdanau-attention-with-coverage kernel for Trainium2 (raw Bass/Bacc).

The reference module applies softmax over a size-1 axis, so every attention
weight is exactly 1.0. The whole network therefore collapses:
  context_vector     = values.sum(axis=1)               # [B, D]
  attention_weights  = ones([B, T, 1])
  coverage_vector    = arange(T) broadcast to [B, T, 1]
The kernel computes exactly that, data-parallel over batch across 8 cores
(4 batches/core).  Per core it is DMA-bound: 16 MB of `values` read from HBM
at ~350 GB/s via 16x 1MB DMAs alternating the two HWDGE queues (SP + ACT);
per-chunk folds run in-place on the DVE under the DMA window, the 128->1
partition fold runs on the PE (indicator-window weights route batch b into
PSUM row b), and the trivial ones/iota outputs go out over SWDGE.
Hand-rolled semaphores (no Tile) minimize entry/exit overhead.
"""

import numpy as np

import concourse.bacc as bacc
import concourse.mybir as mybir
from concourse.bass_utils import run_bass_kernel_spmd

B, T, D = 32, 1024, 1024
N_CORES = 8
BL = B // N_CORES
F32 = mybir.dt.float32
P = 128
NCH = T // P

_nc = None


def _build():
    nc = bacc.Bacc("TRN2", target_bir_lowering=False, num_swdge_queues=1)

    vals = nc.dram_tensor("values", [BL, T, D], F32, kind="ExternalInput")
    ctx_o = nc.dram_tensor("ctx", [BL, D], F32, kind="ExternalOutput")
    aw_o = nc.dram_tensor("aw", [BL, T, 1], F32, kind="ExternalOutput")
    cov_o = nc.dram_tensor("cov", [BL, T, 1], F32, kind="ExternalOutput")

    C = 2 * D  # chunk: [128, 2048] f32 = 1 MB

    from contextlib import ExitStack
    es = ExitStack()
    with (
        es,
        nc.sbuf_tensor([P, 16, C], F32) as ck,       # 16 chunk buffers
        nc.sbuf_tensor([P, 2 * BL], F32) as wz,      # indicator window
        nc.sbuf_tensor([P, BL * T // P], F32) as ones_t,
        nc.sbuf_tensor([P, BL, NCH], mybir.dt.int32) as iota_t,
        nc.sbuf_tensor([P, BL, NCH], F32) as cov_t,
        nc.sbuf_tensor([BL, D], F32) as ctx_sb,
        nc.psum_tensor([BL, D], F32) as acc,
        nc.semaphore("dve_sem") as dve_sem,
        nc.semaphore("pe_sem") as pe_sem,
        nc.semaphore("pool_dma") as pool_dma,
        nc.semaphore("pool_cmp") as pool_cmp,
        nc.Block() as block,
    ):
        # one sem per in-flight DMA: the sim has no HWDGE-FIFO model, so
        # shared-sem increments from concurrent DMAs would be flagged racy
        sp_sems = [es.enter_context(nc.semaphore(f"sp{j}")) for j in range(8)]
        act_sems = [es.enter_context(nc.semaphore(f"act{j}")) for j in range(8)]
        # chunk index: batch b gets chunks 4b..4b+3; SP carries even chunks,
        # ACT odd chunks, interleaved in batch order.
        def src(b):
            return vals.ap()[b].rearrange("(p n) d -> p (n d)", p=P)

        @block.sync
        def _(sync):
            for b in range(BL):
                for i in (0, 2):
                    sync.dma_start(
                        out=ck[:, 4 * b + i, :], in_=src(b)[:, i * C : (i + 1) * C]
                    ).then_inc(sp_sems[2 * b + i // 2], 16)
            for j in range(8):
                sync.wait_ge(sp_sems[j], 16)

        @block.scalar
        def _(scalar):
            for b in range(BL):
                for i in (1, 3):
                    scalar.dma_start(
                        out=ck[:, 4 * b + i, :], in_=src(b)[:, i * C : (i + 1) * C]
                    ).then_inc(act_sems[2 * b + i // 2], 16)
            for j in range(8):
                scalar.wait_ge(act_sems[j], 16)

        @block.vector
        def _(vector):
            # every DVE op incs dve_sem; dependent ops wait on the count
            # (the DVE pipeline has no same-engine hazard interlock)
            vector.memset(wz[:, 0:BL], 0.0).then_inc(dve_sem, 1)
            vector.memset(wz[:, BL + 1 : 2 * BL], 0.0).then_inc(dve_sem, 1)
            vector.memset(wz[:, BL : BL + 1], 1.0).then_inc(dve_sem, 1)
            for b in range(BL):
                k = 4 * b
                n0 = 3 + 7 * b
                # per-chunk halves-folds (in place, left half accumulates)
                vector.wait_ge(sp_sems[2 * b], 16)
                vector.tensor_add(
                    out=ck[:, k + 0, :D], in0=ck[:, k + 0, :D], in1=ck[:, k + 0, D:]
                ).then_inc(dve_sem, 1)
                vector.wait_ge(act_sems[2 * b], 16)
                vector.tensor_add(
                    out=ck[:, k + 1, :D], in0=ck[:, k + 1, :D], in1=ck[:, k + 1, D:]
                ).then_inc(dve_sem, 1)
                vector.wait_ge(dve_sem, n0 + 2)
                vector.tensor_add(
                    out=ck[:, k + 0, :D], in0=ck[:, k + 0, :D], in1=ck[:, k + 1, :D]
                ).then_inc(dve_sem, 1)
                vector.wait_ge(sp_sems[2 * b + 1], 16)
                vector.tensor_add(
                    out=ck[:, k + 2, :D], in0=ck[:, k + 2, :D], in1=ck[:, k + 2, D:]
                ).then_inc(dve_sem, 1)
                vector.wait_ge(act_sems[2 * b + 1], 16)
                vector.tensor_add(
                    out=ck[:, k + 3, :D], in0=ck[:, k + 3, :D], in1=ck[:, k + 3, D:]
                ).then_inc(dve_sem, 1)
                vector.wait_ge(dve_sem, n0 + 5)
                vector.tensor_add(
                    out=ck[:, k + 2, :D], in0=ck[:, k + 2, :D], in1=ck[:, k + 3, :D]
                ).then_inc(dve_sem, 1)
                vector.wait_ge(dve_sem, n0 + 6)
                vector.tensor_add(
                    out=ck[:, k + 0, :D], in0=ck[:, k + 0, :D], in1=ck[:, k + 2, :D]
                ).then_inc(dve_sem, 1)
            # psum -> sbuf after the PE finishes
            vector.wait_ge(pe_sem, 1)
            vector.tensor_copy(out=ctx_sb[:], in_=acc[:]).then_inc(dve_sem, 1)

        @block.tensor
        def _(tensor):
            for b in range(BL):
                tensor.wait_ge(dve_sem, 10 + 7 * b)
                lhsT = wz[:, BL - b : 2 * BL - b]
                a1 = ck[:, 4 * b, :D]
                tensor.matmul(
                    acc[:, 0:512], lhsT, a1[:, 0:512],
                    start=(b == 0), stop=(b == BL - 1),
                )
                mm = tensor.matmul(
                    acc[:, 512:1024], lhsT, a1[:, 512:1024],
                    start=(b == 0), stop=(b == BL - 1),
                )
                if b == BL - 1:
                    mm.then_inc(pe_sem, 1)

        @block.gpsimd
        def _(gpsimd):
            # trivial outputs, all SWDGE; DMA execution is async so each DMA
            # sem-gates on its producer even on the same engine
            gpsimd.memset(ones_t[:], 1.0).then_inc(pool_cmp, 1)
            gpsimd.iota(
                iota_t[:], pattern=[[0, BL], [1, NCH]], base=0, channel_multiplier=NCH
            ).then_inc(pool_cmp, 1)
            gpsimd.wait_ge(pool_cmp, 2)
            gpsimd.tensor_copy(out=cov_t[:], in_=iota_t[:]).then_inc(pool_cmp, 1)
            gpsimd.wait_ge(pool_cmp, 2)
            gpsimd.dma_start(
                out=aw_o.ap().flatten_outer_dims().rearrange("(p n) c -> p (n c)", p=P),
                in_=ones_t[:],
            ).then_inc(pool_dma, 16)
            gpsimd.wait_ge(pool_cmp, 3)
            gpsimd.dma_start(
                out=cov_o.ap().rearrange("b (p n) c -> p b (n c)", p=P),
                in_=cov_t[:],
            ).then_inc(pool_dma, 16)
            gpsimd.wait_ge(dve_sem, 32)
            gpsimd.dma_start(out=ctx_o.ap(), in_=ctx_sb[:]).then_inc(pool_dma, 16)
            gpsimd.wait_ge(pool_dma, 48)

    nc.compile()
    return nc


def _get_nc():
    global _nc
    if _nc is None:
        _nc = _build()
    return _nc


def kernel(**inputs) -> tuple:
    values = np.ascontiguousarray(np.asarray(inputs["values"], dtype=np.float32))
    assert values.shape == (B, T, D), values.shape

    nc = _get_nc()
    in_maps = [
        {"values": np.ascontiguousarray(values[c * BL : (c + 1) * BL])}
        for c in range(N_CORES)
    ]
    res = run_bass_kernel_spmd(nc, in_maps, core_ids=list(range(N_CORES)))

    ctx = np.concatenate([r["ctx"] for r in res.results], axis=0)
    aw = np.concatenate([r["aw"] for r in res.results], axis=0)
    cov = np.concatenate([r["cov"] for r in res.results], axis=0)
    return ctx, aw, cov
